# revision 23
# baseline (speedup 1.0000x reference)
"""GroupedESN Trainium2 kernel.

Problem: E=8 echo-state networks, batch B=16, T=512 steps, reservoir R=512,
input D=8.  h_{t+1} = (1-a) h_t + a tanh(W_in x_t + W_res h_t), output is the
final state concatenated over ESNs -> [B, E*R].

Sharding: one ESN per NeuronCore (8 cores).  Inside a core the recurrence is
sequential over T; per step the tensor engine re-ingests W (fp16 stationary,
fast-weight-load) as 16 [128,128] chunks.

State substitution (a folded into W, so per-core program is data-independent):
  g = h / a,  W'' = a * W_res,  c = 1 - a
  g_{t+1} = c g_t + tanh(u_t + W'' g_t)
Split g = sigma + tau so the only serial op between steps is the tanh:
  tau_{t+1}   = tanh(u_t + W'' sigma_t + W'' tau_t)     (scalar engine)
  sigma_{t+1} = c (sigma_t + tau_t)                     (vector, off-chain)

Default mode 'one': psum bank (t//8)%8 holds 8 steps' pre-activations
(col = rc*128 + (t%8)*16 + b), so ONE 64-col tanh per step reads all 4 rc
groups and the serial cycle is mm-burst -> tanh -> mm-burst: one Activation
roundtrip (~390 ns) + one 16-matmul LDW-dominated burst (~640 ns) per step.
Measured per-step components (HW, slope timing): Act roundtrip 388 ns
(serial tanh-chain probe), full step ~1100 ns.  Rejected by measurement:
fp8 weights (no LDW speedup, 24x rel err), psum m-carry (Act/DVE same-bank
collision + has_written), batch-split chains (PE-bound), DoubleRow (FWL off).
"""

import os
import sys

import numpy as np

for _p in ("/opt/trn_rl_repo", "/root/.axon_site/_ro/trn_rl_repo"):
    if _p not in sys.path and os.path.isdir(_p):
        sys.path.append(_p)

E, B, T, R, D = 8, 16, 512, 512, 8
NCORES = 8
BLK = 32          # timesteps per psum block
NBLK = T // BLK   # 16

# mm modes: 'sumap'  - one matmul per weight chunk, rhs=[sigma|tau], out AP
#                      broadcast so both halves accumulate into same 16 cols
#           '2mm'    - two matmuls per chunk (relies on walrus LDW dedupe)
#           'g'      - single g state, blend on critical path
#           'one'    - sumap rhs trick, but ONE 64-col tanh per step (psum
#                      bank holds all 4 rc groups per step) so the serial
#                      cycle pays a single Activation roundtrip, not two
#           'tau'    - DEAD END, kept for reference: carrying m_t = W''g_t
#                      in PSUM needs a second reader of the step's psum bank
#                      (fatal Act/DVE same-bank collision on TRN2) or a
#                      non-PE psum write under PE accumulation (has_written
#                      bits make later matmuls overwrite it)
#           'one8'   - 'one' with fp8e4m3 W''; measured SAME speed as fp16
#                      on HW (FWL fp8 load rate did not double) at 24x the
#                      rel err (1.1e-2 vs 4.6e-4) -> not used
MODE = os.environ.get("ESN_MODE", "one")


def _build_nc(mode=MODE, timesteps=T, reps=1):
    """reps>1 wraps the whole body in a tc.For_i loop: the NEFF executes the
    (idempotent) kernel body `reps` times back-to-back.  Used only by the
    timing harness to amortize host dispatch; the output equals reps=1."""
    from contextlib import ExitStack

    import concourse.bass as bass  # noqa: F401
    import concourse.tile as tile
    from concourse import bacc, mybir

    f16 = mybir.dt.float16
    f32 = mybir.dt.float32
    AF = mybir.ActivationFunctionType
    OP = mybir.AluOpType

    nc = bacc.Bacc(
        "TRN2",
        target_bir_lowering=False,
        debug=False,
        enable_asserts=False,
        num_devices=NCORES,
    )
    f8 = mybir.dt.float8e4
    wdt = f8 if mode == "one8" else f16
    wt_d = nc.dram_tensor("wt", [128, 2048], wdt, kind="ExternalInput").ap()
    win_d = nc.dram_tensor("win", [8, 512], f16, kind="ExternalInput").ap()
    xt_d = nc.dram_tensor("xt", [8, T * 16], f16, kind="ExternalInput").ap()
    ca_d = nc.dram_tensor("ca", [128, 2], f32, kind="ExternalInput").ap()
    out_d = nc.dram_tensor("out", [128, 64], f32, kind="ExternalOutput").ap()

    nblk = timesteps // BLK
    assert timesteps % BLK == 0

    with tile.TileContext(nc) as tc, ExitStack() as ctx:
        const = ctx.enter_context(tc.tile_pool(name="const", bufs=1))
        wt = const.tile([128, 2048], wdt, tag="wt")
        win = const.tile([8, 512], f16, tag="win")
        xt = const.tile([8, T * 16], f16, tag="xt")
        ca = const.tile([128, 2], f32, tag="ca")
        nc.gpsimd.dma_start(wt[:], wt_d[:])
        nc.gpsimd.dma_start(win[:], win_d[:])
        nc.gpsimd.dma_start(xt[:], xt_d[:])
        nc.gpsimd.dma_start(ca[:], ca_d[:])

        statep = ctx.enter_context(tc.tile_pool(name="state", bufs=1))
        tmpp = ctx.enter_context(tc.tile_pool(name="tmp", bufs=2))
        psp = ctx.enter_context(tc.tile_pool(name="ps", bufs=1, space="PSUM"))
        ps = [psp.tile([128, 512], f32, name=f"ps{i}", tag=f"ps{i}") for i in range(8)]

        c_ap = ca[:, 0:1]
        a_ap = ca[:, 1:2]

        if mode in ("sumap", "2mm", "one", "one8", "onec"):
            st = [statep.tile([128, 128], f16, name=f"st{i}", tag=f"st{i}") for i in range(2)]
        elif mode == "tau":
            taut = [statep.tile([128, 64], f16, name=f"ta{i}", tag=f"ta{i}") for i in range(2)]
            gacc = statep.tile([128, 64], f16, name="gacc", tag="gacc")
        else:  # 'g'
            gt = [statep.tile([128, 64], f16, name=f"gt{i}", tag=f"g{i}") for i in range(2)]
            tt = [statep.tile([128, 64], f16, name=f"tt{i}", tag=f"t{i}") for i in range(2)]

        def bank(blk_i, half, par):
            return ps[(blk_i % 2) * 4 + half * 2 + par]

        def xin_mms(k):
            # project x into psum banks for block k: u in fp32 psum
            for rcp in range(2):          # lhsT chunk; rc-major for LDW reuse
                for half in range(2):
                    rc = half * 2 + rcp
                    for par in range(2):
                        nc.tensor.matmul(
                            bank(k, half, par)[:, rcp * 256:(rcp + 1) * 256],
                            win[:, rc * 128:(rc + 1) * 128],
                            xt[:, k * 512 + par * 256: k * 512 + (par + 1) * 256],
                            start=(rcp == 0),
                            stop=False,
                            skip_group_check=True,
                        )

        # feasible order: qcA-consumers early, qcB-consumers late, A-half
        # (rc0,rc1) groups complete by position 9
        MM_ORDER = [(0, 0), (0, 1), (1, 0), (1, 1), (2, 0), (3, 0),
                    (0, 2), (0, 3), (1, 2), (1, 3), (2, 1), (3, 1),
                    (2, 2), (2, 3), (3, 2), (3, 3)]
        # last position of each rc group in MM_ORDER
        RC_LAST = {0: 7, 1: 9, 2: 13, 3: 15}

        def body_one():
            # bank (t//8)%8 holds steps' pre-activations.  Layout 'one':
            # col = rc*128 + (t%8)*16 + b (tanh reads 4 strided chunks);
            # 'onec': col = (t%8)*64 + rc*16 + b (tanh reads one contiguous
            # 64-col block -- PSUM cachelines are 8B, strided APs pay).
            contig = mode == "onec"
            nblk1 = timesteps // 8
            assert timesteps % 8 == 0
            nc.vector.memset(st[0][:], 0.0)

            def xin_one(k):
                # start=True only on the first mm: each start=True clears the
                # has_written bits of the WHOLE bank, so later openers would
                # wipe the earlier rc regions' bits and the step matmuls
                # would overwrite u instead of accumulating.
                bk = ps[k % 8]
                for rc in range(4):
                    if contig:
                        out_ap = bk[:].rearrange(
                            "p (i r b) -> p r i b", i=8, r=4)[:, rc, :, :]
                    else:
                        out_ap = bk[:, rc * 128:(rc + 1) * 128]
                    nc.tensor.matmul(
                        out_ap,
                        win[:, rc * 128:(rc + 1) * 128],
                        xt[:, k * 128:(k + 1) * 128],
                        start=(rc == 0), stop=False, skip_group_check=True,
                    )

            xin_one(0)
            if nblk1 > 1:
                xin_one(1)
            for t in range(timesteps):
                bk = ps[(t // 8) % 8]
                idx = t % 8
                if t % 8 == 0 and t // 8 + 2 < nblk1:
                    xin_one(t // 8 + 2)
                so, sn = st[t % 2], st[(t + 1) % 2]
                so4 = so[:].rearrange("p (q s) -> p q s", q=4)
                sn4 = sn[:].rearrange("p (q s) -> p q s", q=4)
                # sigma' = c*(sigma+tau), off the serial chain
                tmp = tmpp.tile([128, 64], f16, tag="tmp")
                tmp3 = tmp[:].rearrange("p (q b) -> p q b", q=4)
                nc.vector.tensor_add(tmp3, so4[:, :, 0:16], so4[:, :, 16:32])
                nc.vector.tensor_scalar_mul(sn4[:, :, 0:16], tmp3, c_ap)
                for rc in range(4):
                    for qc in range(4):
                        colb = (idx * 64 + rc * 16) if contig else (rc * 128 + idx * 16)
                        outr = bk[:, colb: colb + 16]
                        out_ap = outr.unsqueeze(1).broadcast_to((128, 2, 16))
                        nc.tensor.matmul(
                            out_ap,
                            wt[:, qc * 512 + rc * 128: qc * 512 + (rc + 1) * 128],
                            so[:, qc * 32:(qc + 1) * 32],
                            start=False, stop=(qc == 3), skip_group_check=True,
                        )
                if contig:
                    src = bk[:].rearrange(
                        "p (i r b) -> p i r b", i=8, r=4)[:, idx, :, :]
                else:
                    src = bk[:].rearrange(
                        "p (r i b) -> p r i b", r=4, i=8)[:, :, idx, :]
                nc.scalar.activation(sn4[:, :, 16:32], src, AF.Tanh)

            # final: h = a * (sigma + tau)
            fin = timesteps % 2
            g32 = tmpp.tile([128, 64], f32, tag="g32")
            sf = st[fin][:].rearrange("p (q s) -> p q s", q=4)
            g3 = g32[:].rearrange("p (q b) -> p q b", q=4)
            nc.vector.tensor_add(g3, sf[:, :, 0:16], sf[:, :, 16:32])
            osb = tmpp.tile([128, 64], f32, tag="osb")
            nc.vector.tensor_scalar_mul(osb[:], g32[:], a_ap)
            nc.gpsimd.dma_start(out_d[:], osb[:])

        def body_tau():
            assert timesteps % 8 == 0
            nc.vector.memset(taut[0][:], 0.0)
            nc.vector.memset(gacc[:], 0.0)

            # step t lives in bank t%8 at column region (t//8)%8: consecutive
            # steps touch different bank tiles, so the carry RMW (writes
            # region t+1) has no false WAR with tanh (reads region t) under
            # the interval-based AP overlap check.
            def region(t):
                bk = ps[t % 8]
                return bk[:].rearrange(
                    "p (r i b) -> p r i b", r=4, i=8)[:, :, (t // 8) % 8, :]

            def xin_step(s):
                bk = ps[s % 8]
                col = ((s // 8) % 8) * 16
                for rc in range(4):
                    nc.tensor.matmul(
                        bk[:, rc * 128 + col: rc * 128 + col + 16],
                        win[:, rc * 128:(rc + 1) * 128],
                        xt[:, s * 16:(s + 1) * 16],
                        start=True, stop=False, skip_group_check=True,
                    )

            for s in range(min(17, timesteps)):
                xin_step(s)
            for t in range(timesteps):
                bk = ps[t % 8]
                col = ((t // 8) % 8) * 16
                cur = region(t)
                if t > 0:
                    # p_t += c * p_{t-1}; reads a closed group, runs in the
                    # prior step's tanh window (off the serial chain).
                    nc.vector.scalar_tensor_tensor(
                        cur, region(t - 1), c_ap, cur, OP.mult, OP.add)
                to = taut[t % 2]
                for rc in range(4):
                    for qc in range(4):
                        nc.tensor.matmul(
                            bk[:, rc * 128 + col: rc * 128 + col + 16],
                            wt[:, qc * 512 + rc * 128: qc * 512 + (rc + 1) * 128],
                            to[:, qc * 16:(qc + 1) * 16],
                            start=False, stop=(qc == 3), skip_group_check=True,
                        )
                # g_t = c*g_{t-1} + tau_t, off-chain
                nc.vector.scalar_tensor_tensor(
                    gacc[:], gacc[:], c_ap, to[:], OP.mult, OP.add)
                if t + 17 < timesteps:
                    xin_step(t + 17)
                tn3 = taut[(t + 1) % 2][:].rearrange("p (q b) -> p q b", q=4)
                nc.scalar.activation(tn3, cur, AF.Tanh)

            # final: g_T = c*g_{T-1} + tau_T;  h = a * g_T
            nc.vector.scalar_tensor_tensor(
                gacc[:], gacc[:], c_ap, taut[timesteps % 2][:], OP.mult, OP.add)
            osb = tmpp.tile([128, 64], f32, tag="osb")
            nc.vector.tensor_scalar_mul(osb[:], gacc[:], a_ap)
            nc.gpsimd.dma_start(out_d[:], osb[:])

        def body():
          if mode in ("one", "one8", "onec"):
              body_one()
              return
          if mode == "tau":
              body_tau()
              return
          if mode in ("sumap", "2mm"):
              nc.vector.memset(st[0][:], 0.0)
          else:
              nc.vector.memset(gt[0][:], 0.0)
          xin_mms(0)
          xin_mms(1)
          for t in range(timesteps):
            blk_i = t // BLK
            par = t % 2
            idx = (t % BLK) // 2
            if t % BLK == 0 and 1 <= blk_i and blk_i + 1 < nblk:
                xin_mms(blk_i + 1)

            if mode in ("sumap", "2mm"):
                so, sn = st[t % 2], st[(t + 1) % 2]
                so4 = so[:].rearrange("p (q s) -> p q s", q=4)
                sn4 = sn[:].rearrange("p (q s) -> p q s", q=4)
                # sigma' = c*(sigma+tau), off critical path
                tmp = tmpp.tile([128, 64], f16, tag="tmp")
                tmp3 = tmp[:].rearrange("p (q b) -> p q b", q=4)
                nc.vector.tensor_add(tmp3, so4[:, :, 0:16], so4[:, :, 16:32])
                nc.vector.tensor_scalar_mul(sn4[:, :, 0:16], tmp3, c_ap)

                def emit_mm(rc, qc):
                    half = rc // 2
                    colb = (rc % 2) * 256 + idx * 16
                    lhsT = wt[:, qc * 512 + rc * 128: qc * 512 + (rc + 1) * 128]
                    stop = RC_LAST[rc] == pos
                    outr = bank(blk_i, half, par)[:, colb:colb + 16]
                    if mode == "sumap":
                        out_ap = outr.unsqueeze(1).broadcast_to((128, 2, 16))
                        nc.tensor.matmul(
                            out_ap, lhsT, so[:, qc * 32:(qc + 1) * 32],
                            start=False, stop=stop, skip_group_check=True)
                    else:
                        nc.tensor.matmul(
                            outr, lhsT, so[:, qc * 32: qc * 32 + 16],
                            start=False, stop=False, skip_group_check=True)
                        nc.tensor.matmul(
                            outr, lhsT, so[:, qc * 32 + 16:(qc + 1) * 32],
                            start=False, stop=stop, skip_group_check=True)

                def emit_tanh(half):
                    b = bank(blk_i, half, par)
                    src = b[:].rearrange("p (r i b) -> p r i b", r=2, i=16)[:, :, idx, :]
                    dst = sn4[:, 2 * half: 2 * half + 2, 16:32]
                    nc.scalar.activation(dst, src, AF.Tanh)

                for pos, (rc, qc) in enumerate(MM_ORDER):
                    emit_mm(rc, qc)
                    if pos == 9:
                        emit_tanh(0)
                emit_tanh(1)
            else:  # 'g' mode
                go, gn = gt[t % 2], gt[(t + 1) % 2]
                tn = tt[(t + 1) % 2]

                for pos, (rc, qc) in enumerate(MM_ORDER):
                    half = rc // 2
                    colb = (rc % 2) * 256 + idx * 16
                    nc.tensor.matmul(
                        bank(blk_i, half, par)[:, colb:colb + 16],
                        wt[:, qc * 512 + rc * 128: qc * 512 + (rc + 1) * 128],
                        go[:, qc * 16:(qc + 1) * 16],
                        start=False, stop=(RC_LAST[rc] == pos),
                        skip_group_check=True)
                    if pos == 9 or pos == 15:
                        half = 0 if pos == 9 else 1
                        b = bank(blk_i, half, par)
                        src = b[:].rearrange("p (r i b) -> p r i b", r=2, i=16)[:, :, idx, :]
                        cols = slice(half * 32, half * 32 + 32)
                        nc.scalar.activation(tn[:, cols], src, AF.Tanh)
                        # g' = c*g + tau   (fused, on chain)
                        nc.vector.scalar_tensor_tensor(
                            gn[:, cols], go[:, cols], c_ap, tn[:, cols],
                            OP.mult, OP.add)

          # final: h = a * (sigma + tau)   [T even -> state in buffer 0]
          fin = timesteps % 2
          g32 = tmpp.tile([128, 64], f32, tag="g32")
          if mode in ("sumap", "2mm"):
              sf = st[fin][:].rearrange("p (q s) -> p q s", q=4)
              g3 = g32[:].rearrange("p (q b) -> p q b", q=4)
              nc.vector.tensor_add(g3, sf[:, :, 0:16], sf[:, :, 16:32])
          else:
              nc.vector.tensor_copy(g32[:], gt[fin][:])
          osb = tmpp.tile([128, 64], f32, tag="osb")
          nc.vector.tensor_scalar_mul(osb[:], g32[:], a_ap)
          nc.gpsimd.dma_start(out_d[:], osb[:])

        if reps == 1:
            body()
        else:
            with tc.For_i(0, reps):
                body()

    nc.compile()
    return nc


def _host_prep(x, W_in, W_res, lr, mode=MODE):
    """Build the 8 per-core input maps."""
    x = np.asarray(x, np.float32)
    W_in = np.asarray(W_in, np.float32)
    W_res = np.asarray(W_res, np.float32)
    lr = np.asarray(lr, np.float32)

    if mode in ("one", "one8", "onec", "tau"):
        # xt[d, t*16 + b] = x[b, t, d]   ('tau': x~_t = x_t - c*x_{t-1},
        # per-core since c varies per ESN -- built below)
        xt = np.ascontiguousarray(
            x.transpose(2, 1, 0).reshape(D, T * B)
        ).astype(np.float16)
    else:
        # xt[d, blk*512 + par*256 + i*16 + b] = x[b, blk*32 + 2*i + par, d]
        xr = x.transpose(2, 1, 0)                 # [D, T, B]
        xr = xr.reshape(D, NBLK, BLK // 2, 2, B)  # [d, blk, i, par, b]
        xt = xr.transpose(0, 1, 3, 2, 4).reshape(D, T * 16)
        xt = np.ascontiguousarray(xt, np.float32).astype(np.float16)

    in_maps = []
    for e in range(NCORES):
        a = np.float32(lr[e])
        wtp = (a * W_res[e]).T                    # [q, r]
        if mode == "one8":
            import ml_dtypes
            w_np_dtype = ml_dtypes.float8_e4m3
        else:
            w_np_dtype = np.float16
        wt = np.ascontiguousarray(
            wtp.reshape(4, 128, 512).transpose(1, 0, 2).reshape(128, 2048)
        ).astype(w_np_dtype)
        win = np.ascontiguousarray(W_in[e].T).astype(np.float16)  # [8, 512]
        ca = np.empty((128, 2), np.float32)
        ca[:, 0] = 1.0 - a
        ca[:, 1] = a
        if mode == "tau":
            xs = x.copy()                         # [B, T, D]
            xs[:, 1:, :] -= (1.0 - a) * x[:, :-1, :]
            xte = np.ascontiguousarray(
                xs.transpose(2, 1, 0).reshape(D, T * B)
            ).astype(np.float16)
        else:
            xte = xt
        in_maps.append({"wt": wt, "win": win, "xt": xte, "ca": ca})
    return in_maps


def _unshard(results):
    out = np.empty((B, E * R), np.float32)
    for e in range(NCORES):
        o = results[e]["out"]                      # [128, 64]
        he = o.reshape(128, 4, 16).transpose(2, 1, 0).reshape(B, R)
        out[:, e * R:(e + 1) * R] = he
    return out


def _run(in_maps, mode=MODE, trace=False, tmpdir=None):
    from concourse import bass_utils

    nc = _build_nc(mode=mode)
    res = bass_utils.run_bass_kernel_spmd(
        nc,
        in_maps,
        core_ids=list(range(NCORES)),
        trace=trace,
        tmpdir=tmpdir,
    )
    return res


def kernel(x, W_in, W_res, lr):
    in_maps = _host_prep(x, W_in, W_res, lr)
    res = _run(in_maps, trace=False)
    return _unshard(res.results)


if __name__ == "__main__":
    rng = np.random.default_rng(0)
    x = rng.normal(size=(B, T, D)).astype(np.float32)
    W_in = rng.normal(size=(E, R, D)).astype(np.float32) * 0.5
    W_res = (rng.normal(size=(E, R, R)) * (rng.random((E, R, R)) < 0.1)).astype(np.float32) * 0.05
    lr = rng.uniform(0.1, 0.5, E).astype(np.float32)
    out = kernel(x, W_in, W_res, lr)
    print("out", out.shape, out.dtype, np.abs(out).max())



# revision 24
# speedup vs baseline: 1.0096x; 1.0096x over previous
"""GroupedESN Trainium2 kernel.

Problem: E=8 echo-state networks, batch B=16, T=512 steps, reservoir R=512,
input D=8.  h_{t+1} = (1-a) h_t + a tanh(W_in x_t + W_res h_t), output is the
final state concatenated over ESNs -> [B, E*R].

Sharding: one ESN per NeuronCore (8 cores).  Inside a core the recurrence is
sequential over T; per step the tensor engine re-ingests W (fp16 stationary,
fast-weight-load) as 16 [128,128] chunks.

State substitution (a folded into W, so per-core program is data-independent):
  g = h / a,  W'' = a * W_res,  c = 1 - a
  g_{t+1} = c g_t + tanh(u_t + W'' g_t)
Split g = sigma + tau so the only serial op between steps is the tanh:
  tau_{t+1}   = tanh(u_t + W'' sigma_t + W'' tau_t)     (scalar engine)
  sigma_{t+1} = c (sigma_t + tau_t)                     (vector, off-chain)

Default mode 'one': psum bank (t//8)%8 holds 8 steps' pre-activations
(col = rc*128 + (t%8)*16 + b), so ONE 64-col tanh per step reads all 4 rc
groups and the serial cycle is mm-burst -> tanh -> mm-burst: one Activation
roundtrip (~390 ns) + one 16-matmul LDW-dominated burst (~640 ns) per step.
Measured per-step components (HW, slope timing): Act roundtrip 388 ns
(serial tanh-chain probe), full step ~1100 ns.  Rejected by measurement:
fp8 weights (no LDW speedup, 24x rel err), psum m-carry (Act/DVE same-bank
collision + has_written), batch-split chains (PE-bound), DoubleRow (FWL off).
"""

import os
import sys

import numpy as np

for _p in ("/opt/trn_rl_repo", "/root/.axon_site/_ro/trn_rl_repo"):
    if _p not in sys.path and os.path.isdir(_p):
        sys.path.append(_p)

E, B, T, R, D = 8, 16, 512, 512, 8
NCORES = 8
BLK = 32          # timesteps per psum block
NBLK = T // BLK   # 16

# mm modes: 'sumap'  - one matmul per weight chunk, rhs=[sigma|tau], out AP
#                      broadcast so both halves accumulate into same 16 cols
#           '2mm'    - two matmuls per chunk (relies on walrus LDW dedupe)
#           'g'      - single g state, blend on critical path
#           'one'    - sumap rhs trick, but ONE 64-col tanh per step (psum
#                      bank holds all 4 rc groups per step) so the serial
#                      cycle pays a single Activation roundtrip, not two
#           'tau'    - DEAD END, kept for reference: carrying m_t = W''g_t
#                      in PSUM needs a second reader of the step's psum bank
#                      (fatal Act/DVE same-bank collision on TRN2) or a
#                      non-PE psum write under PE accumulation (has_written
#                      bits make later matmuls overwrite it)
#           'one8'   - 'one' with fp8e4m3 W''; measured SAME speed as fp16
#                      on HW (FWL fp8 load rate did not double) at 24x the
#                      rel err (1.1e-2 vs 4.6e-4) -> not used
MODE = os.environ.get("ESN_MODE", "one")


def _build_nc(mode=MODE, timesteps=T, reps=1):
    """reps>1 wraps the whole body in a tc.For_i loop: the NEFF executes the
    (idempotent) kernel body `reps` times back-to-back.  Used only by the
    timing harness to amortize host dispatch; the output equals reps=1."""
    from contextlib import ExitStack

    import concourse.bass as bass  # noqa: F401
    import concourse.tile as tile
    from concourse import bacc, mybir

    f16 = mybir.dt.float16
    f32 = mybir.dt.float32
    AF = mybir.ActivationFunctionType
    OP = mybir.AluOpType

    nc = bacc.Bacc(
        "TRN2",
        target_bir_lowering=False,
        debug=False,
        enable_asserts=False,
        num_devices=NCORES,
    )
    f8 = mybir.dt.float8e4
    wdt = f8 if mode == "one8" else f16
    wt_d = nc.dram_tensor("wt", [128, 2048], wdt, kind="ExternalInput").ap()
    win_d = nc.dram_tensor("win", [8, 512], f16, kind="ExternalInput").ap()
    xt_d = nc.dram_tensor("xt", [8, T * 16], f16, kind="ExternalInput").ap()
    ca_d = nc.dram_tensor("ca", [128, 2], f32, kind="ExternalInput").ap()
    out_d = nc.dram_tensor("out", [128, 64], f32, kind="ExternalOutput").ap()

    nblk = timesteps // BLK
    assert timesteps % BLK == 0

    with tile.TileContext(nc) as tc, ExitStack() as ctx:
        const = ctx.enter_context(tc.tile_pool(name="const", bufs=1))
        wt = const.tile([128, 2048], wdt, tag="wt")
        win = const.tile([8, 512], f16, tag="win")
        xt = const.tile([8, T * 16], f16, tag="xt")
        ca = const.tile([128, 2], f32, tag="ca")
        nc.gpsimd.dma_start(wt[:], wt_d[:])
        nc.gpsimd.dma_start(win[:], win_d[:])
        nc.gpsimd.dma_start(xt[:], xt_d[:])
        nc.gpsimd.dma_start(ca[:], ca_d[:])

        statep = ctx.enter_context(tc.tile_pool(name="state", bufs=1))
        tmpp = ctx.enter_context(tc.tile_pool(name="tmp", bufs=2))
        psp = ctx.enter_context(tc.tile_pool(name="ps", bufs=1, space="PSUM"))
        ps = [psp.tile([128, 512], f32, name=f"ps{i}", tag=f"ps{i}") for i in range(8)]

        c_ap = ca[:, 0:1]
        a_ap = ca[:, 1:2]

        if mode in ("sumap", "2mm", "one", "one8", "onec", "onel"):
            st = [statep.tile([128, 128], f16, name=f"st{i}", tag=f"st{i}") for i in range(2)]
        elif mode == "tau":
            taut = [statep.tile([128, 64], f16, name=f"ta{i}", tag=f"ta{i}") for i in range(2)]
            gacc = statep.tile([128, 64], f16, name="gacc", tag="gacc")
        else:  # 'g'
            gt = [statep.tile([128, 64], f16, name=f"gt{i}", tag=f"g{i}") for i in range(2)]
            tt = [statep.tile([128, 64], f16, name=f"tt{i}", tag=f"t{i}") for i in range(2)]

        def bank(blk_i, half, par):
            return ps[(blk_i % 2) * 4 + half * 2 + par]

        def xin_mms(k):
            # project x into psum banks for block k: u in fp32 psum
            for rcp in range(2):          # lhsT chunk; rc-major for LDW reuse
                for half in range(2):
                    rc = half * 2 + rcp
                    for par in range(2):
                        nc.tensor.matmul(
                            bank(k, half, par)[:, rcp * 256:(rcp + 1) * 256],
                            win[:, rc * 128:(rc + 1) * 128],
                            xt[:, k * 512 + par * 256: k * 512 + (par + 1) * 256],
                            start=(rcp == 0),
                            stop=False,
                            skip_group_check=True,
                        )

        # feasible order: qcA-consumers early, qcB-consumers late, A-half
        # (rc0,rc1) groups complete by position 9
        MM_ORDER = [(0, 0), (0, 1), (1, 0), (1, 1), (2, 0), (3, 0),
                    (0, 2), (0, 3), (1, 2), (1, 3), (2, 1), (3, 1),
                    (2, 2), (2, 3), (3, 2), (3, 3)]
        # last position of each rc group in MM_ORDER
        RC_LAST = {0: 7, 1: 9, 2: 13, 3: 15}

        def body_one():
            # bank (t//8)%8 holds steps' pre-activations.  Layout 'one':
            # col = rc*128 + (t%8)*16 + b (tanh reads 4 strided chunks);
            # 'onec': col = (t%8)*64 + rc*16 + b (tanh reads one contiguous
            # 64-col block -- PSUM cachelines are 8B, strided APs pay).
            contig = mode == "onec"
            nblk1 = timesteps // 8
            assert timesteps % 8 == 0
            nc.vector.memset(st[0][:], 0.0)

            def xin_one(k):
                # start=True only on the first mm: each start=True clears the
                # has_written bits of the WHOLE bank, so later openers would
                # wipe the earlier rc regions' bits and the step matmuls
                # would overwrite u instead of accumulating.
                bk = ps[k % 8]
                for rc in range(4):
                    if contig:
                        out_ap = bk[:].rearrange(
                            "p (i r b) -> p r i b", i=8, r=4)[:, rc, :, :]
                    else:
                        out_ap = bk[:, rc * 128:(rc + 1) * 128]
                    nc.tensor.matmul(
                        out_ap,
                        win[:, rc * 128:(rc + 1) * 128],
                        xt[:, k * 128:(k + 1) * 128],
                        start=(rc == 0), stop=False, skip_group_check=True,
                    )

            xin_one(0)
            if nblk1 > 1:
                xin_one(1)
            xin_late = mode == "onel"
            for t in range(timesteps):
                bk = ps[(t // 8) % 8]
                idx = t % 8
                if not xin_late and t % 8 == 0 and t // 8 + 2 < nblk1:
                    xin_one(t // 8 + 2)
                so, sn = st[t % 2], st[(t + 1) % 2]
                so4 = so[:].rearrange("p (q s) -> p q s", q=4)
                sn4 = sn[:].rearrange("p (q s) -> p q s", q=4)
                # sigma' = c*(sigma+tau), off the serial chain
                tmp = tmpp.tile([128, 64], f16, tag="tmp")
                tmp3 = tmp[:].rearrange("p (q b) -> p q b", q=4)
                nc.vector.tensor_add(tmp3, so4[:, :, 0:16], so4[:, :, 16:32])
                nc.vector.tensor_scalar_mul(sn4[:, :, 0:16], tmp3, c_ap)
                for rc in range(4):
                    for qc in range(4):
                        colb = (idx * 64 + rc * 16) if contig else (rc * 128 + idx * 16)
                        outr = bk[:, colb: colb + 16]
                        out_ap = outr.unsqueeze(1).broadcast_to((128, 2, 16))
                        nc.tensor.matmul(
                            out_ap,
                            wt[:, qc * 512 + rc * 128: qc * 512 + (rc + 1) * 128],
                            so[:, qc * 32:(qc + 1) * 32],
                            start=False, stop=(qc == 3), skip_group_check=True,
                        )
                if xin_late and t % 8 == 5 and t // 8 + 2 < nblk1:
                    # volley AFTER this step's mms: PE streams it during the
                    # tanh window, staying mid-burst when the sem arrives
                    xin_one(t // 8 + 2)
                if contig:
                    src = bk[:].rearrange(
                        "p (i r b) -> p i r b", i=8, r=4)[:, idx, :, :]
                else:
                    src = bk[:].rearrange(
                        "p (r i b) -> p r i b", r=4, i=8)[:, :, idx, :]
                nc.scalar.activation(sn4[:, :, 16:32], src, AF.Tanh)

            # final: h = a * (sigma + tau)
            fin = timesteps % 2
            g32 = tmpp.tile([128, 64], f32, tag="g32")
            sf = st[fin][:].rearrange("p (q s) -> p q s", q=4)
            g3 = g32[:].rearrange("p (q b) -> p q b", q=4)
            nc.vector.tensor_add(g3, sf[:, :, 0:16], sf[:, :, 16:32])
            osb = tmpp.tile([128, 64], f32, tag="osb")
            nc.vector.tensor_scalar_mul(osb[:], g32[:], a_ap)
            nc.gpsimd.dma_start(out_d[:], osb[:])

        def body_tau():
            assert timesteps % 8 == 0
            nc.vector.memset(taut[0][:], 0.0)
            nc.vector.memset(gacc[:], 0.0)

            # step t lives in bank t%8 at column region (t//8)%8: consecutive
            # steps touch different bank tiles, so the carry RMW (writes
            # region t+1) has no false WAR with tanh (reads region t) under
            # the interval-based AP overlap check.
            def region(t):
                bk = ps[t % 8]
                return bk[:].rearrange(
                    "p (r i b) -> p r i b", r=4, i=8)[:, :, (t // 8) % 8, :]

            def xin_step(s):
                bk = ps[s % 8]
                col = ((s // 8) % 8) * 16
                for rc in range(4):
                    nc.tensor.matmul(
                        bk[:, rc * 128 + col: rc * 128 + col + 16],
                        win[:, rc * 128:(rc + 1) * 128],
                        xt[:, s * 16:(s + 1) * 16],
                        start=True, stop=False, skip_group_check=True,
                    )

            for s in range(min(17, timesteps)):
                xin_step(s)
            for t in range(timesteps):
                bk = ps[t % 8]
                col = ((t // 8) % 8) * 16
                cur = region(t)
                if t > 0:
                    # p_t += c * p_{t-1}; reads a closed group, runs in the
                    # prior step's tanh window (off the serial chain).
                    nc.vector.scalar_tensor_tensor(
                        cur, region(t - 1), c_ap, cur, OP.mult, OP.add)
                to = taut[t % 2]
                for rc in range(4):
                    for qc in range(4):
                        nc.tensor.matmul(
                            bk[:, rc * 128 + col: rc * 128 + col + 16],
                            wt[:, qc * 512 + rc * 128: qc * 512 + (rc + 1) * 128],
                            to[:, qc * 16:(qc + 1) * 16],
                            start=False, stop=(qc == 3), skip_group_check=True,
                        )
                # g_t = c*g_{t-1} + tau_t, off-chain
                nc.vector.scalar_tensor_tensor(
                    gacc[:], gacc[:], c_ap, to[:], OP.mult, OP.add)
                if t + 17 < timesteps:
                    xin_step(t + 17)
                tn3 = taut[(t + 1) % 2][:].rearrange("p (q b) -> p q b", q=4)
                nc.scalar.activation(tn3, cur, AF.Tanh)

            # final: g_T = c*g_{T-1} + tau_T;  h = a * g_T
            nc.vector.scalar_tensor_tensor(
                gacc[:], gacc[:], c_ap, taut[timesteps % 2][:], OP.mult, OP.add)
            osb = tmpp.tile([128, 64], f32, tag="osb")
            nc.vector.tensor_scalar_mul(osb[:], gacc[:], a_ap)
            nc.gpsimd.dma_start(out_d[:], osb[:])

        def body():
          if mode in ("one", "one8", "onec", "onel"):
              body_one()
              return
          if mode == "tau":
              body_tau()
              return
          if mode in ("sumap", "2mm"):
              nc.vector.memset(st[0][:], 0.0)
          else:
              nc.vector.memset(gt[0][:], 0.0)
          xin_mms(0)
          xin_mms(1)
          for t in range(timesteps):
            blk_i = t // BLK
            par = t % 2
            idx = (t % BLK) // 2
            if t % BLK == 0 and 1 <= blk_i and blk_i + 1 < nblk:
                xin_mms(blk_i + 1)

            if mode in ("sumap", "2mm"):
                so, sn = st[t % 2], st[(t + 1) % 2]
                so4 = so[:].rearrange("p (q s) -> p q s", q=4)
                sn4 = sn[:].rearrange("p (q s) -> p q s", q=4)
                # sigma' = c*(sigma+tau), off critical path
                tmp = tmpp.tile([128, 64], f16, tag="tmp")
                tmp3 = tmp[:].rearrange("p (q b) -> p q b", q=4)
                nc.vector.tensor_add(tmp3, so4[:, :, 0:16], so4[:, :, 16:32])
                nc.vector.tensor_scalar_mul(sn4[:, :, 0:16], tmp3, c_ap)

                def emit_mm(rc, qc):
                    half = rc // 2
                    colb = (rc % 2) * 256 + idx * 16
                    lhsT = wt[:, qc * 512 + rc * 128: qc * 512 + (rc + 1) * 128]
                    stop = RC_LAST[rc] == pos
                    outr = bank(blk_i, half, par)[:, colb:colb + 16]
                    if mode == "sumap":
                        out_ap = outr.unsqueeze(1).broadcast_to((128, 2, 16))
                        nc.tensor.matmul(
                            out_ap, lhsT, so[:, qc * 32:(qc + 1) * 32],
                            start=False, stop=stop, skip_group_check=True)
                    else:
                        nc.tensor.matmul(
                            outr, lhsT, so[:, qc * 32: qc * 32 + 16],
                            start=False, stop=False, skip_group_check=True)
                        nc.tensor.matmul(
                            outr, lhsT, so[:, qc * 32 + 16:(qc + 1) * 32],
                            start=False, stop=stop, skip_group_check=True)

                def emit_tanh(half):
                    b = bank(blk_i, half, par)
                    src = b[:].rearrange("p (r i b) -> p r i b", r=2, i=16)[:, :, idx, :]
                    dst = sn4[:, 2 * half: 2 * half + 2, 16:32]
                    nc.scalar.activation(dst, src, AF.Tanh)

                for pos, (rc, qc) in enumerate(MM_ORDER):
                    emit_mm(rc, qc)
                    if pos == 9:
                        emit_tanh(0)
                emit_tanh(1)
            else:  # 'g' mode
                go, gn = gt[t % 2], gt[(t + 1) % 2]
                tn = tt[(t + 1) % 2]

                for pos, (rc, qc) in enumerate(MM_ORDER):
                    half = rc // 2
                    colb = (rc % 2) * 256 + idx * 16
                    nc.tensor.matmul(
                        bank(blk_i, half, par)[:, colb:colb + 16],
                        wt[:, qc * 512 + rc * 128: qc * 512 + (rc + 1) * 128],
                        go[:, qc * 16:(qc + 1) * 16],
                        start=False, stop=(RC_LAST[rc] == pos),
                        skip_group_check=True)
                    if pos == 9 or pos == 15:
                        half = 0 if pos == 9 else 1
                        b = bank(blk_i, half, par)
                        src = b[:].rearrange("p (r i b) -> p r i b", r=2, i=16)[:, :, idx, :]
                        cols = slice(half * 32, half * 32 + 32)
                        nc.scalar.activation(tn[:, cols], src, AF.Tanh)
                        # g' = c*g + tau   (fused, on chain)
                        nc.vector.scalar_tensor_tensor(
                            gn[:, cols], go[:, cols], c_ap, tn[:, cols],
                            OP.mult, OP.add)

          # final: h = a * (sigma + tau)   [T even -> state in buffer 0]
          fin = timesteps % 2
          g32 = tmpp.tile([128, 64], f32, tag="g32")
          if mode in ("sumap", "2mm"):
              sf = st[fin][:].rearrange("p (q s) -> p q s", q=4)
              g3 = g32[:].rearrange("p (q b) -> p q b", q=4)
              nc.vector.tensor_add(g3, sf[:, :, 0:16], sf[:, :, 16:32])
          else:
              nc.vector.tensor_copy(g32[:], gt[fin][:])
          osb = tmpp.tile([128, 64], f32, tag="osb")
          nc.vector.tensor_scalar_mul(osb[:], g32[:], a_ap)
          nc.gpsimd.dma_start(out_d[:], osb[:])

        if reps == 1:
            body()
        else:
            with tc.For_i(0, reps):
                body()

    nc.compile()
    return nc


def _host_prep(x, W_in, W_res, lr, mode=MODE):
    """Build the 8 per-core input maps."""
    x = np.asarray(x, np.float32)
    W_in = np.asarray(W_in, np.float32)
    W_res = np.asarray(W_res, np.float32)
    lr = np.asarray(lr, np.float32)

    if mode in ("one", "one8", "onec", "onel", "tau"):
        # xt[d, t*16 + b] = x[b, t, d]   ('tau': x~_t = x_t - c*x_{t-1},
        # per-core since c varies per ESN -- built below)
        xt = np.ascontiguousarray(
            x.transpose(2, 1, 0).reshape(D, T * B)
        ).astype(np.float16)
    else:
        # xt[d, blk*512 + par*256 + i*16 + b] = x[b, blk*32 + 2*i + par, d]
        xr = x.transpose(2, 1, 0)                 # [D, T, B]
        xr = xr.reshape(D, NBLK, BLK // 2, 2, B)  # [d, blk, i, par, b]
        xt = xr.transpose(0, 1, 3, 2, 4).reshape(D, T * 16)
        xt = np.ascontiguousarray(xt, np.float32).astype(np.float16)

    in_maps = []
    for e in range(NCORES):
        a = np.float32(lr[e])
        wtp = (a * W_res[e]).T                    # [q, r]
        if mode == "one8":
            import ml_dtypes
            w_np_dtype = ml_dtypes.float8_e4m3
        else:
            w_np_dtype = np.float16
        wt = np.ascontiguousarray(
            wtp.reshape(4, 128, 512).transpose(1, 0, 2).reshape(128, 2048)
        ).astype(w_np_dtype)
        win = np.ascontiguousarray(W_in[e].T).astype(np.float16)  # [8, 512]
        ca = np.empty((128, 2), np.float32)
        ca[:, 0] = 1.0 - a
        ca[:, 1] = a
        if mode == "tau":
            xs = x.copy()                         # [B, T, D]
            xs[:, 1:, :] -= (1.0 - a) * x[:, :-1, :]
            xte = np.ascontiguousarray(
                xs.transpose(2, 1, 0).reshape(D, T * B)
            ).astype(np.float16)
        else:
            xte = xt
        in_maps.append({"wt": wt, "win": win, "xt": xte, "ca": ca})
    return in_maps


def _unshard(results):
    out = np.empty((B, E * R), np.float32)
    for e in range(NCORES):
        o = results[e]["out"]                      # [128, 64]
        he = o.reshape(128, 4, 16).transpose(2, 1, 0).reshape(B, R)
        out[:, e * R:(e + 1) * R] = he
    return out


def _run(in_maps, mode=MODE, trace=False, tmpdir=None):
    from concourse import bass_utils

    nc = _build_nc(mode=mode)
    res = bass_utils.run_bass_kernel_spmd(
        nc,
        in_maps,
        core_ids=list(range(NCORES)),
        trace=trace,
        tmpdir=tmpdir,
    )
    return res


def kernel(x, W_in, W_res, lr):
    in_maps = _host_prep(x, W_in, W_res, lr)
    res = _run(in_maps, trace=False)
    return _unshard(res.results)


if __name__ == "__main__":
    rng = np.random.default_rng(0)
    x = rng.normal(size=(B, T, D)).astype(np.float32)
    W_in = rng.normal(size=(E, R, D)).astype(np.float32) * 0.5
    W_res = (rng.normal(size=(E, R, R)) * (rng.random((E, R, R)) < 0.1)).astype(np.float32) * 0.05
    lr = rng.uniform(0.1, 0.5, E).astype(np.float32)
    out = kernel(x, W_in, W_res, lr)
    print("out", out.shape, out.dtype, np.abs(out).max())

